# revision 36
# baseline (speedup 1.0000x reference)
"""Causal attention (RoPE, 16 heads, L=2048, H=2048) on 8 trn2 NeuronCores.

Sharding: tensor-parallel over heads. Core i handles heads 2i, 2i+1
(d=128 each): column-parallel q/k/v projections, row-parallel o_proj,
host-side sum of the 8 partial outputs.

v2: single fused instruction stream; block i interleaves projection
chunk lc=i+1 with attention q-chunk qc=i and o_proj of qc=i-1 so the
ACT-bound exp work hides under PE-bound projection matmuls.
  - Q^T/K^T in [d, L] layout (weight-stationary, bf16 in, N=512);
    RoPE on DVE reading PSUM directly.
  - V in natural [L, d] layout (x-stationary, N=256) - no transposes.
  - Causal mask preloaded into PSUM as additive -1e30 (DVE copy), the
    S^T matmul accumulates onto it with start=False; exp gives exact 0.
  - Softmax denominators: gpsimd accumulates exp tiles elementwise
    (ptsum), a single ones-matmul per (head, q-chunk) reduces over
    partitions; DVE reciprocal + multiply normalizes into ot (aliasing
    the dead qt tile).
  - o_proj per 128-row q-tile into a contiguous [128, 2048] buffer,
    one 1MB output DMA each, round-robin across queues at the tail.
"""
import numpy as np

L = 2048
H = 2048
NH = 16
D = 128          # head dim
NCORES = 8
HPC = NH // NCORES   # heads per core = 2
ROPE_BASE = 10000.0
KC = H // 128        # 16 contraction chunks
LCN = 4              # L chunks of 512
QCN = 4              # q chunks of 512

_CACHE = {}


def _rope_tables():
    inv_freq = 1.0 / (ROPE_BASE ** (np.arange(0, D, 2, dtype=np.float32) / D))
    t = np.arange(L, dtype=np.float32)
    freqs = np.outer(t, inv_freq).astype(np.float32)          # [L, D/2]
    emb = np.concatenate([freqs, freqs], axis=-1)             # [L, D]
    cos = np.cos(emb).astype(np.float32)                      # [L, D]
    sin = np.sin(emb).astype(np.float32)
    cosT = np.ascontiguousarray(cos.T)                        # [D, L]
    sinT = np.ascontiguousarray(sin.T)
    sinTs = sinT.copy()
    sinTs[: D // 2] = -sinT[: D // 2]                         # sign-folded
    # partition-swapped so DVE operand base partitions match:
    # sinsw[p] = sinTs[(p+64) % 128]
    sinsw = np.concatenate([sinTs[D // 2:], sinTs[: D // 2]], axis=0)
    return cosT, np.ascontiguousarray(sinsw)


def _causal_masks():
    # additive masks for the 4 diagonal-block variants (j = kt - 4*qc):
    # keep (0.0) iff col - row - 128*j >= 0, else -1e30.
    # layout [128 rows(k), 4 j, 512 cols(q)]
    row = np.arange(128)[:, None]
    col = np.arange(512)[None, :]
    m = np.zeros((128, 4, 512), dtype=np.float32)
    for j in range(4):
        m[:, j, :] = np.where(col - row - 128 * j >= 0, 0.0, -1.0e30)
    return m


def _build_nc():
    import concourse.bacc as bacc
    import concourse.mybir as mybir
    from concourse import tile
    from contextlib import ExitStack

    f32 = mybir.dt.float32
    f32r = mybir.dt.float32r
    bf16 = mybir.dt.bfloat16
    AF = mybir.ActivationFunctionType
    OP = mybir.AluOpType

    nc = bacc.Bacc("TRN2", target_bir_lowering=False, debug=False)

    # weights host-rearranged to partition-major so DMAs are contiguous
    xT_d = nc.dram_tensor("xT", (KC, 128, L), bf16, kind="ExternalInput")
    wq_d = nc.dram_tensor("wqT", (128, KC, HPC * D), bf16, kind="ExternalInput")
    wk_d = nc.dram_tensor("wkT", (128, KC, HPC * D), bf16, kind="ExternalInput")
    wv_d = nc.dram_tensor("wvT", (128, KC, HPC * D), bf16, kind="ExternalInput")
    wo_d = nc.dram_tensor("woP", (128, HPC, H), bf16, kind="ExternalInput")
    cos_d = nc.dram_tensor("cosT", (D, L), bf16, kind="ExternalInput")
    sin_d = nc.dram_tensor("sinTs", (D, L), bf16, kind="ExternalInput")
    msk_d = nc.dram_tensor("masks", (128, 4, 512), bf16, kind="ExternalInput")
    ones_d = nc.dram_tensor("ones", (128, 128), f32r, kind="ExternalInput")
    out_d = nc.dram_tensor("out", (L, H), bf16, kind="ExternalOutput")

    with tile.TileContext(nc) as tc, ExitStack() as top:
        per = top.enter_context(tc.tile_pool(name="per", bufs=1))

        wq_sb = per.tile([128, KC, HPC * D], bf16)
        wk_sb = per.tile([128, KC, HPC * D], bf16)
        wv_sb = per.tile([128, KC, HPC * D], bf16)
        wo_sb = per.tile([128, HPC, H], bf16)
        cos_sb = per.tile([128, L], bf16)
        sin_sb = per.tile([128, L], bf16)
        msk_sb = per.tile([128, 4, 512], bf16)
        ones_sb = per.tile([128, 128], f32r)
        qt_sb = [[per.tile([128, 512], f32r, name=f"qt{h}_{c}")
                  for c in range(QCN)] for h in range(HPC)]
        # normalized O^T in bf16 (o_proj runs fully bf16)
        ot_sb = [[per.tile([128, 512], bf16, name=f"ot{h}_{c}")
                  for c in range(QCN)] for h in range(HPC)]
        kt_sb = [[per.tile([128, 512], f32r, name=f"kt{h}_{c}")
                  for c in range(LCN)] for h in range(HPC)]
        # natural V: [l within 128-tile, lt, d of both heads]
        v_sb = per.tile([128, KC, HPC * D], bf16, name="v")
        ptsum = [per.tile([128, 512], f32r, name=f"ptsum{h}") for h in range(HPC)]
        # all of x^T stays resident: 16 chunks x [128, 2048] bf16 = 8MB,
        # loaded once with 4KB partition lines (small lines run ~45GB/s);
        # one tile per chunk so dependencies stay per-kc
        xt_k = [per.tile([128, L], bf16, name=f"xt{kc}") for kc in range(KC)]

        ptpool = top.enter_context(tc.tile_pool(name="pt", bufs=3))
        tpool = top.enter_context(tc.tile_pool(name="tmp", bufs=1))
        obpool = top.enter_context(tc.tile_pool(name="ob", bufs=2))
        psum_pools = {}

        # ---------- initial DMAs ----------
        # Per-queue DMA runs ~90GB/s and there are only 3 queues, so every
        # transfer is ordered by its consumption deadline.  x chunks load
        # as column halves (2KB lines): lc0/lc1 half first, lc2/lc3 later.
        for a, b in ((0, 2), (2, 6), (6, 10), (10, 16)):
            nc.scalar.dma_start(wq_sb[:, a:b, :], wq_d[:, a:b, :])
        nc.scalar.dma_start(wv_sb[:, 0:8, :], wv_d[:, 0:8, :])
        nc.scalar.dma_start(wv_sb[:, 8:16, :], wv_d[:, 8:16, :])
        nc.scalar.dma_start(wo_sb[:], wo_d[:])

        def emit_x_dma(kcs, eng, half):
            c = slice(0, 1024) if half == 0 else slice(1024, 2048)
            for kc in kcs:
                eng.dma_start(xt_k[kc][:, c], xT_d[kc, :, c])

        # ---------------- thunk builders ----------------
        def rope_evict(ps, dst, lc):
            cs = slice(lc * 512, (lc + 1) * 512)
            t1 = tpool.tile([128, 512], f32, tag="t1")
            t2 = tpool.tile([128, 512], f32, tag="t2")
            nc.vector.tensor_tensor(t2[:], ps[:], cos_sb[:, cs], OP.mult)
            nc.vector.tensor_tensor(
                t1[0:64, :], ps[64:128, :], sin_sb[64:128, cs], OP.mult)
            nc.vector.tensor_tensor(
                t1[64:128, :], ps[0:64, :], sin_sb[0:64, cs], OP.mult)
            nc.vector.tensor_tensor(dst[:], t1[:], t2[:], OP.add)

        def qk_chain(pool, tag, w_sb, lc, h, dst):
            """16 matmul thunks accumulating one head's Q^T/K^T chunk."""
            ps = pool.tile([128, 512], f32, tag=tag, name="ps")
            thunks = []
            for kc in range(KC):
                def mm(kc=kc, ps=ps, w_sb=w_sb, h=h, lc=lc):
                    nc.tensor.matmul(
                        ps[:], w_sb[:, kc, h * D:(h + 1) * D],
                        xt_k[kc][:, lc * 512:(lc + 1) * 512],
                        start=(kc == 0), stop=(kc == KC - 1))
                thunks.append(mm)
            thunks[-1] = (thunks[-1], lambda ps=ps, dst=dst, lc=lc:
                          rope_evict(ps, dst, lc))
            return thunks

        def v_chain(pool, tag, lc, lt):
            ps = pool.tile([128, 512], f32, tag=tag, name="ps")
            thunks = []
            for kc in range(KC):
                def mm(kc=kc, ps=ps, lt=lt, lc=lc):
                    c0 = lc * 512 + lt * 128
                    nc.tensor.matmul(
                        ps[:, 0:HPC * D],
                        xt_k[kc][:, c0:c0 + 128],
                        wv_sb[:, kc, :],
                        start=(kc == 0), stop=(kc == KC - 1))
                thunks.append(mm)
            def ev(ps=ps, lt=lt, lc=lc):
                nc.scalar.copy(v_sb[:, lc * 4 + lt, :], ps[:, 0:HPC * D])
            thunks[-1] = (thunks[-1], ev)
            return thunks

        def proj_thunks(lc):
            """128 PE thunks for chunk lc: Q0,K0,Q1,K1 then V (2 psum bufs).

            Q/K alternate so each chain's RoPE eviction has a full chain of
            slack before its psum slot is reused."""
            pj = psum_pools["pj"]
            thunks = []
            thunks += qk_chain(pj, "pj", wq_sb, lc, 0, qt_sb[0][lc])
            thunks += qk_chain(pj, "pj", wk_sb, lc, 0, kt_sb[0][lc])
            thunks += qk_chain(pj, "pj", wq_sb, lc, 1, qt_sb[1][lc])
            thunks += qk_chain(pj, "pj", wk_sb, lc, 1, kt_sb[1][lc])
            for lt in range(4):
                thunks += v_chain(pj, "pj", lc, lt)
            return thunks

        def att_units(qc):
            """n_kt+2 units: pipelined S/exp then PV/accum one kt behind."""
            n_kt = 4 * qc + 4
            units = []
            pts = {}    # (kt, h) -> pt tile
            acc_p = psum_pools["acc"]
            sps_p = psum_pools["sps"]
            sums = [acc_p.tile([128, 512], f32, tag=f"o{h}", name=f"ops{h}")
                    for h in range(HPC)]

            def step(u, qc=qc, n_kt=n_kt):
                if u < n_kt:
                    kt = u
                    diag = kt >= 4 * qc
                    # diagonal tiles only have valid cols >= 128j; compute S
                    # and exp on [c0:512] (c0 capped at 256: fp32r needs
                    # N>=256) and zero the dead pt columns on DVE
                    c0 = min(128 * (kt - 4 * qc), 256) if diag else 0
                    for h in range(HPC):
                        s = sps_p.tile([128, 512], f32, tag="s")
                        if diag:
                            nc.vector.tensor_copy(
                                s[:, c0:], msk_sb[:, kt - 4 * qc, c0:])
                        nc.tensor.matmul(
                            s[:, c0:],
                            kt_sb[h][kt // 4][:, (kt % 4) * 128:(kt % 4 + 1) * 128],
                            qt_sb[h][qc][:, c0:],
                            start=not diag, stop=True,
                            skip_group_check=diag)
                        pt = ptpool.tile([128, 512], bf16, tag=f"pt{h}")
                        if c0:
                            nc.vector.memset(pt[:, 0:c0], 0.0)
                        nc.scalar.activation(pt[:, c0:], s[:, c0:], AF.Exp)
                        pts[(kt, h)] = pt
                if u >= 1:
                    kt = u - 1
                    for h in range(HPC):
                        nc.tensor.matmul(
                            sums[h][:], v_sb[:, kt, h * D:(h + 1) * D],
                            pts[(kt, h)][:],
                            start=(kt == 0), stop=(kt == n_kt - 1))
                    for h in range(HPC):
                        # h0 on DVE, h1 on gpsimd; last kt both on DVE (it
                        # sits on the flush critical path and DVE is faster)
                        eng = nc.vector if (h == 0 or kt == n_kt - 1) \
                            else nc.gpsimd
                        if kt == 0:
                            eng.tensor_copy(ptsum[h][:], pts[(kt, h)][:])
                        else:
                            eng.tensor_tensor(
                                ptsum[h][:], ptsum[h][:], pts[(kt, h)][:],
                                OP.add)
                        del pts[(kt, h)]

            def flush(qc=qc):
                for h in range(HPC):
                    den = sps_p.tile([128, 512], f32, tag="s")
                    nc.tensor.matmul(den[:], ones_sb[:], ptsum[h][:],
                                     start=True, stop=True)
                    rc = tpool.tile([128, 512], f32, tag="rc")
                    nc.vector.reciprocal_approx_fast(rc[:], den[:])
                    # last chunk sliced so o_proj can start on slice 0 early
                    nsl = 4 if qc == QCN - 1 else 1
                    for sl in range(nsl):
                        c = slice(sl * 512 // nsl, (sl + 1) * 512 // nsl)
                        nc.vector.tensor_tensor(ot_sb[h][qc][:, c],
                                                sums[h][:, c], rc[:, c],
                                                OP.mult)

            for u in range(n_kt + 1):
                units.append(lambda u=u: step(u))
            units.append(flush)
            return units

        ob_tiles = {}

        def oproj_units(qc, tailq=False):
            """16 units: (qt4, hcn) -> 2 matmuls + evict; DMA per qt4."""
            units = []
            for qt4 in range(4):
                for hcn in range(4):
                    def grp(qc=qc, qt4=qt4, hcn=hcn, tailq=tailq):
                        if hcn == 0:
                            ob_tiles[(qc, qt4)] = obpool.tile(
                                [128, H], bf16, tag="ob", name="ob")
                        ob = ob_tiles[(qc, qt4)]
                        po = psum_pools["pop"].tile([128, 512], f32, tag="po",
                                                    name="po")
                        for h in range(HPC):
                            nc.tensor.matmul(
                                po[:], ot_sb[h][qc][:, qt4 * 128:(qt4 + 1) * 128],
                                wo_sb[:, h, hcn * 512:(hcn + 1) * 512],
                                start=(h == 0), stop=(h == HPC - 1))
                        dst = ob[:, hcn * 512:(hcn + 1) * 512]
                        if hcn % 2 == 0:
                            nc.scalar.copy(dst, po[:])
                        else:
                            nc.vector.tensor_copy(dst, po[:])
                        if hcn == 3:
                            qt = qc * 4 + qt4
                            engs = ([nc.sync, nc.gpsimd, nc.scalar]
                                    if tailq else [nc.sync, nc.gpsimd])
                            eng = engs[qt4 % len(engs)]
                            eng.dma_start(
                                out_d[qt * 128:(qt + 1) * 128, :], ob[:])
                            del ob_tiles[(qc, qt4)]
                    units.append(grp)
            return units

        def emit_interleaved(pe_thunks, unit_list):
            """Spread units evenly among the PE thunk stream."""
            n_t, n_u = len(pe_thunks), len(unit_list)
            ui = 0
            for i, th in enumerate(pe_thunks):
                while ui < n_u and ui * (n_t + 1) <= i * n_u:
                    unit_list[ui]()
                    ui += 1
                if isinstance(th, tuple):
                    th[0]()
                    th[1]()
                else:
                    th()
            while ui < n_u:
                unit_list[ui]()
                ui += 1

        # ---------------- emission ----------------
        # the gpsimd DMA queue measures ~1.5x the sync queue's rate, so it
        # carries 10 of the 16 x chunks; interleaved so arrivals track the
        # chains' in-order kc consumption
        g_kcs = [0, 1, 2, 4, 5, 7, 8, 10, 11, 13]
        s_kcs = [3, 6, 9, 12, 14, 15]
        nc.sync.dma_start(wk_sb[:, 0:2, :], wk_d[:, 0:2, :])
        emit_x_dma([3], nc.sync, 0)
        nc.sync.dma_start(wk_sb[:, 2:6, :], wk_d[:, 2:6, :])
        emit_x_dma([6], nc.sync, 0)
        nc.sync.dma_start(wk_sb[:, 6:10, :], wk_d[:, 6:10, :])
        emit_x_dma([9], nc.sync, 0)
        nc.sync.dma_start(wk_sb[:, 10:16, :], wk_d[:, 10:16, :])
        emit_x_dma([12, 14, 15], nc.sync, 0)
        emit_x_dma(s_kcs, nc.sync, 1)
        emit_x_dma([0, 1, 2], nc.gpsimd, 0)
        nc.gpsimd.dma_start(cos_sb[:], cos_d[:])
        nc.gpsimd.dma_start(sin_sb[:], sin_d[:])
        emit_x_dma([4, 5, 7, 8], nc.gpsimd, 0)
        nc.gpsimd.dma_start(msk_sb[:], msk_d[:])
        nc.gpsimd.dma_start(ones_sb[:], ones_d[:])
        emit_x_dma([10, 11, 13], nc.gpsimd, 0)
        emit_x_dma(g_kcs, nc.gpsimd, 1)

        # lc0 in its own wide psum pool (closed before the steady-state
        # pools open): Q0/Q1/K0/K1 interleaved per kc, then V lt-major.
        with ExitStack() as lc0_scope:
            lc0_p = lc0_scope.enter_context(
                tc.tile_pool(name="lc0", bufs=8, space="PSUM"))
            qk = [(wq_sb, 0, qt_sb[0][0]), (wq_sb, 1, qt_sb[1][0]),
                  (wk_sb, 0, kt_sb[0][0]), (wk_sb, 1, kt_sb[1][0])]
            chains = [qk_chain(lc0_p, "l0", w, 0, h, dst) for w, h, dst in qk]
            for kc in range(KC):
                for ch in chains:
                    th = ch[kc]
                    if isinstance(th, tuple):
                        th[0]()
                        th[1]()
                    else:
                        th()
            for lt in range(4):
                for th in v_chain(lc0_p, "l0", 0, lt):
                    if isinstance(th, tuple):
                        th[0]()
                        th[1]()
                    else:
                        th()

        psum_pools["pj"] = top.enter_context(
            tc.tile_pool(name="pj", bufs=2, space="PSUM"))
        psum_pools["sps"] = top.enter_context(
            tc.tile_pool(name="sps", bufs=2, space="PSUM"))
        psum_pools["acc"] = top.enter_context(
            tc.tile_pool(name="acc", bufs=1, space="PSUM"))
        psum_pools["pop"] = top.enter_context(
            tc.tile_pool(name="pop", bufs=2, space="PSUM"))

        for i in range(QCN):
            units = att_units(i)
            if i >= 1:
                units = _merge(units, oproj_units(i - 1))
            if i < 3:
                emit_interleaved(proj_thunks(i + 1), units)
            else:
                emit_interleaved([], units)
        emit_interleaved([], oproj_units(3, tailq=True))

    nc.compile()
    return nc


def _merge(a, b):
    """Round-robin merge of two unit lists, proportionally."""
    out = []
    ia = ib = 0
    n = len(a) + len(b)
    for i in range(n):
        if ia * len(b) <= ib * len(a) and ia < len(a):
            out.append(a[ia]); ia += 1
        elif ib < len(b):
            out.append(b[ib]); ib += 1
        else:
            out.append(a[ia]); ia += 1
    return out


def _prep_inputs(x, Wq, Wk, Wv, Wo):
    import ml_dtypes
    bf16 = ml_dtypes.bfloat16
    xT = np.ascontiguousarray(x.reshape(L, H).T).astype(bf16).reshape(KC, 128, L)
    cosT, sinTs = _rope_tables()
    masks = _causal_masks()
    ones = np.ones((128, 128), dtype=np.float32)
    scale = np.float32(1.0 / np.sqrt(D))
    def pmajor(w):     # [H, 256] -> [128, KC, 256] partition-major
        return np.ascontiguousarray(
            w.reshape(KC, 128, HPC * D).transpose(1, 0, 2))
    in_maps = []
    for i in range(NCORES):
        rs = slice(i * HPC * D, (i + 1) * HPC * D)
        in_maps.append({
            "xT": xT,
            "wqT": pmajor((Wq[rs].T * scale).astype(bf16)),
            "wkT": pmajor(Wk[rs].T.astype(bf16)),
            "wvT": pmajor(Wv[rs].T.astype(bf16)),
            "woP": np.ascontiguousarray(
                Wo[:, rs].T.reshape(HPC, 128, H).transpose(1, 0, 2)).astype(bf16),
            "cosT": cosT.astype(bf16),
            "sinTs": sinTs.astype(bf16),
            "masks": masks.astype(bf16),
            "ones": ones,
        })
    return in_maps


def run(x, Wq, Wk, Wv, Wo, trace=False):
    from concourse.bass_utils import run_bass_kernel_spmd
    if "nc" not in _CACHE:
        _CACHE["nc"] = _build_nc()
    nc = _CACHE["nc"]
    in_maps = _prep_inputs(np.asarray(x), np.asarray(Wq), np.asarray(Wk),
                           np.asarray(Wv), np.asarray(Wo))
    res = run_bass_kernel_spmd(nc, in_maps, core_ids=list(range(NCORES)),
                               trace=trace)
    acc = np.zeros((L, H), dtype=np.float64)
    for r in res.results:
        acc += r["out"].astype(np.float64)
    return acc.astype(np.float32).reshape(1, L, H), res


def kernel(x, Wq, Wk, Wv, Wo):
    out, _ = run(x, Wq, Wk, Wv, Wo)
    return out


# revision 37
# speedup vs baseline: 1.0375x; 1.0375x over previous
"""Causal attention (RoPE, 16 heads, L=2048, H=2048) on 8 trn2 NeuronCores.

Sharding: tensor-parallel over heads. Core i handles heads 2i, 2i+1
(d=128 each): column-parallel q/k/v projections, row-parallel o_proj,
host-side sum of the 8 partial outputs.

v2: single fused instruction stream; block i interleaves projection
chunk lc=i+1 with attention q-chunk qc=i and o_proj of qc=i-1 so the
ACT-bound exp work hides under PE-bound projection matmuls.
  - Q^T/K^T in [d, L] layout (weight-stationary, bf16 in, N=512);
    RoPE on DVE reading PSUM directly.
  - V in natural [L, d] layout (x-stationary, N=256) - no transposes.
  - Causal mask preloaded into PSUM as additive -1e30 (DVE copy), the
    S^T matmul accumulates onto it with start=False; exp gives exact 0.
  - Softmax denominators: gpsimd accumulates exp tiles elementwise
    (ptsum), a single ones-matmul per (head, q-chunk) reduces over
    partitions; DVE reciprocal + multiply normalizes into ot (aliasing
    the dead qt tile).
  - o_proj per 128-row q-tile into a contiguous [128, 2048] buffer,
    one 1MB output DMA each, round-robin across queues at the tail.
"""
import numpy as np

L = 2048
H = 2048
NH = 16
D = 128          # head dim
NCORES = 8
HPC = NH // NCORES   # heads per core = 2
ROPE_BASE = 10000.0
KC = H // 128        # 16 contraction chunks
LCN = 4              # L chunks of 512
QCN = 4              # q chunks of 512

_CACHE = {}


def _rope_tables():
    inv_freq = 1.0 / (ROPE_BASE ** (np.arange(0, D, 2, dtype=np.float32) / D))
    t = np.arange(L, dtype=np.float32)
    freqs = np.outer(t, inv_freq).astype(np.float32)          # [L, D/2]
    emb = np.concatenate([freqs, freqs], axis=-1)             # [L, D]
    cos = np.cos(emb).astype(np.float32)                      # [L, D]
    sin = np.sin(emb).astype(np.float32)
    cosT = np.ascontiguousarray(cos.T)                        # [D, L]
    sinT = np.ascontiguousarray(sin.T)
    sinTs = sinT.copy()
    sinTs[: D // 2] = -sinT[: D // 2]                         # sign-folded
    # partition-swapped so DVE operand base partitions match:
    # sinsw[p] = sinTs[(p+64) % 128]
    sinsw = np.concatenate([sinTs[D // 2:], sinTs[: D // 2]], axis=0)
    return cosT, np.ascontiguousarray(sinsw)


def _causal_masks():
    # additive masks for the 4 diagonal-block variants (j = kt - 4*qc):
    # keep (0.0) iff col - row - 128*j >= 0, else -1e30.
    # layout [128 rows(k), 4 j, 512 cols(q)]
    row = np.arange(128)[:, None]
    col = np.arange(512)[None, :]
    m = np.zeros((128, 4, 512), dtype=np.float32)
    for j in range(4):
        m[:, j, :] = np.where(col - row - 128 * j >= 0, 0.0, -1.0e30)
    return m


def _build_nc():
    import concourse.bacc as bacc
    import concourse.mybir as mybir
    from concourse import tile
    from contextlib import ExitStack

    f32 = mybir.dt.float32
    f32r = mybir.dt.float32r
    bf16 = mybir.dt.bfloat16
    AF = mybir.ActivationFunctionType
    OP = mybir.AluOpType

    nc = bacc.Bacc("TRN2", target_bir_lowering=False, debug=False)

    # weights host-rearranged to partition-major so DMAs are contiguous
    xT_d = nc.dram_tensor("xT", (KC, 128, L), bf16, kind="ExternalInput")
    wq_d = nc.dram_tensor("wqT", (128, KC, HPC * D), bf16, kind="ExternalInput")
    wk_d = nc.dram_tensor("wkT", (128, KC, HPC * D), bf16, kind="ExternalInput")
    wv_d = nc.dram_tensor("wvT", (128, KC, HPC * D), bf16, kind="ExternalInput")
    wo_d = nc.dram_tensor("woP", (128, HPC, H), bf16, kind="ExternalInput")
    cos_d = nc.dram_tensor("cosT", (D, L), bf16, kind="ExternalInput")
    sin_d = nc.dram_tensor("sinTs", (D, L), bf16, kind="ExternalInput")
    msk_d = nc.dram_tensor("masks", (128, 4, 512), bf16, kind="ExternalInput")
    ones_d = nc.dram_tensor("ones", (128, 128), f32r, kind="ExternalInput")
    out_d = nc.dram_tensor("out", (L, H), bf16, kind="ExternalOutput")

    with tile.TileContext(nc) as tc, ExitStack() as top:
        per = top.enter_context(tc.tile_pool(name="per", bufs=1))

        wq_sb = per.tile([128, KC, HPC * D], bf16)
        wk_sb = per.tile([128, KC, HPC * D], bf16)
        wv_sb = per.tile([128, KC, HPC * D], bf16)
        wo_sb = per.tile([128, HPC, H], bf16)
        cos_sb = per.tile([128, L], bf16)
        sin_sb = per.tile([128, L], bf16)
        msk_sb = per.tile([128, 4, 512], bf16)
        ones_sb = per.tile([128, 128], f32r)
        qt_sb = [[per.tile([128, 512], bf16, name=f"qt{h}_{c}")
                  for c in range(QCN)] for h in range(HPC)]
        # normalized O^T in bf16 (o_proj runs fully bf16)
        ot_sb = [[per.tile([128, 512], bf16, name=f"ot{h}_{c}")
                  for c in range(QCN)] for h in range(HPC)]
        kt_sb = [[per.tile([128, 512], bf16, name=f"kt{h}_{c}")
                  for c in range(LCN)] for h in range(HPC)]
        # natural V: [l within 128-tile, lt, d of both heads]
        v_sb = per.tile([128, KC, HPC * D], bf16, name="v")
        ptsum = [per.tile([128, 512], f32r, name=f"ptsum{h}") for h in range(HPC)]
        # all of x^T stays resident: 16 chunks x [128, 2048] bf16 = 8MB,
        # loaded once with 4KB partition lines (small lines run ~45GB/s);
        # one tile per chunk so dependencies stay per-kc
        xt_k = [per.tile([128, L], bf16, name=f"xt{kc}") for kc in range(KC)]

        ptpool = top.enter_context(tc.tile_pool(name="pt", bufs=3))
        tpool = top.enter_context(tc.tile_pool(name="tmp", bufs=1))
        obpool = top.enter_context(tc.tile_pool(name="ob", bufs=2))
        psum_pools = {}

        # ---------- initial DMAs ----------
        # Per-queue DMA runs ~90GB/s and there are only 3 queues, so every
        # transfer is ordered by its consumption deadline.  x chunks load
        # as column halves (2KB lines): lc0/lc1 half first, lc2/lc3 later.
        for a, b in ((0, 2), (2, 6), (6, 10), (10, 16)):
            nc.scalar.dma_start(wq_sb[:, a:b, :], wq_d[:, a:b, :])
            nc.scalar.dma_start(wk_sb[:, a:b, :], wk_d[:, a:b, :])
        nc.scalar.dma_start(wv_sb[:, 0:8, :], wv_d[:, 0:8, :])
        nc.scalar.dma_start(wv_sb[:, 8:16, :], wv_d[:, 8:16, :])
        nc.scalar.dma_start(wo_sb[:], wo_d[:])

        def emit_x_dma(kcs, eng, half):
            c = slice(0, 1024) if half == 0 else slice(1024, 2048)
            for kc in kcs:
                eng.dma_start(xt_k[kc][:, c], xT_d[kc, :, c])

        # ---------------- thunk builders ----------------
        def rope_evict(ps, dst, lc):
            cs = slice(lc * 512, (lc + 1) * 512)
            t1 = tpool.tile([128, 512], f32, tag="t1")
            t2 = tpool.tile([128, 512], f32, tag="t2")
            nc.vector.tensor_tensor(t2[:], ps[:], cos_sb[:, cs], OP.mult)
            nc.vector.tensor_tensor(
                t1[0:64, :], ps[64:128, :], sin_sb[64:128, cs], OP.mult)
            nc.vector.tensor_tensor(
                t1[64:128, :], ps[0:64, :], sin_sb[0:64, cs], OP.mult)
            nc.vector.tensor_tensor(dst[:], t1[:], t2[:], OP.add)

        def qk_chain(pool, tag, w_sb, lc, h, dst):
            """16 matmul thunks accumulating one head's Q^T/K^T chunk."""
            ps = pool.tile([128, 512], f32, tag=tag, name="ps")
            thunks = []
            for kc in range(KC):
                def mm(kc=kc, ps=ps, w_sb=w_sb, h=h, lc=lc):
                    nc.tensor.matmul(
                        ps[:], w_sb[:, kc, h * D:(h + 1) * D],
                        xt_k[kc][:, lc * 512:(lc + 1) * 512],
                        start=(kc == 0), stop=(kc == KC - 1))
                thunks.append(mm)
            thunks[-1] = (thunks[-1], lambda ps=ps, dst=dst, lc=lc:
                          rope_evict(ps, dst, lc))
            return thunks

        def v_chain(pool, tag, lc, lt):
            ps = pool.tile([128, 512], f32, tag=tag, name="ps")
            thunks = []
            for kc in range(KC):
                def mm(kc=kc, ps=ps, lt=lt, lc=lc):
                    c0 = lc * 512 + lt * 128
                    nc.tensor.matmul(
                        ps[:, 0:HPC * D],
                        xt_k[kc][:, c0:c0 + 128],
                        wv_sb[:, kc, :],
                        start=(kc == 0), stop=(kc == KC - 1))
                thunks.append(mm)
            def ev(ps=ps, lt=lt, lc=lc):
                nc.scalar.copy(v_sb[:, lc * 4 + lt, :], ps[:, 0:HPC * D])
            thunks[-1] = (thunks[-1], ev)
            return thunks

        def proj_thunks(lc):
            """128 PE thunks for chunk lc: Q0,K0,Q1,K1 then V (2 psum bufs).

            Q/K alternate so each chain's RoPE eviction has a full chain of
            slack before its psum slot is reused."""
            pj = psum_pools["pj"]
            thunks = []
            thunks += qk_chain(pj, "pj", wq_sb, lc, 0, qt_sb[0][lc])
            thunks += qk_chain(pj, "pj", wk_sb, lc, 0, kt_sb[0][lc])
            thunks += qk_chain(pj, "pj", wq_sb, lc, 1, qt_sb[1][lc])
            thunks += qk_chain(pj, "pj", wk_sb, lc, 1, kt_sb[1][lc])
            for lt in range(4):
                thunks += v_chain(pj, "pj", lc, lt)
            return thunks

        def att_units(qc):
            """n_kt+2 units: pipelined S/exp then PV/accum one kt behind."""
            n_kt = 4 * qc + 4
            units = []
            pts = {}    # (kt, h) -> pt tile
            acc_p = psum_pools["acc"]
            sps_p = psum_pools["sps"]
            sums = [acc_p.tile([128, 512], f32, tag=f"o{h}", name=f"ops{h}")
                    for h in range(HPC)]

            def step(u, qc=qc, n_kt=n_kt):
                if u < n_kt:
                    kt = u
                    diag = kt >= 4 * qc
                    # diagonal tiles only have valid cols >= 128j; compute S
                    # and exp on [c0:512] (c0 capped at 256: fp32r needs
                    # N>=256) and zero the dead pt columns on DVE
                    c0 = min(128 * (kt - 4 * qc), 256) if diag else 0
                    for h in range(HPC):
                        s = sps_p.tile([128, 512], f32, tag="s")
                        if diag:
                            nc.vector.tensor_copy(
                                s[:, c0:], msk_sb[:, kt - 4 * qc, c0:])
                        nc.tensor.matmul(
                            s[:, c0:],
                            kt_sb[h][kt // 4][:, (kt % 4) * 128:(kt % 4 + 1) * 128],
                            qt_sb[h][qc][:, c0:],
                            start=not diag, stop=True,
                            skip_group_check=diag)
                        pt = ptpool.tile([128, 512], bf16, tag=f"pt{h}")
                        if c0:
                            nc.vector.memset(pt[:, 0:c0], 0.0)
                        nc.scalar.activation(pt[:, c0:], s[:, c0:], AF.Exp)
                        pts[(kt, h)] = pt
                if u >= 1:
                    kt = u - 1
                    for h in range(HPC):
                        nc.tensor.matmul(
                            sums[h][:], v_sb[:, kt, h * D:(h + 1) * D],
                            pts[(kt, h)][:],
                            start=(kt == 0), stop=(kt == n_kt - 1))
                    for h in range(HPC):
                        # h0 on DVE, h1 on gpsimd; last kt both on DVE (it
                        # sits on the flush critical path and DVE is faster)
                        eng = nc.vector if (h == 0 or kt == n_kt - 1) \
                            else nc.gpsimd
                        if kt == 0:
                            eng.tensor_copy(ptsum[h][:], pts[(kt, h)][:])
                        else:
                            eng.tensor_tensor(
                                ptsum[h][:], ptsum[h][:], pts[(kt, h)][:],
                                OP.add)
                        del pts[(kt, h)]

            def flush(qc=qc):
                for h in range(HPC):
                    den = sps_p.tile([128, 512], f32, tag="s")
                    nc.tensor.matmul(den[:], ones_sb[:], ptsum[h][:],
                                     start=True, stop=True)
                    rc = tpool.tile([128, 512], f32, tag="rc")
                    nc.vector.reciprocal_approx_fast(rc[:], den[:])
                    # last chunk sliced so o_proj can start on slice 0 early
                    nsl = 4 if qc == QCN - 1 else 1
                    for sl in range(nsl):
                        c = slice(sl * 512 // nsl, (sl + 1) * 512 // nsl)
                        nc.vector.tensor_tensor(ot_sb[h][qc][:, c],
                                                sums[h][:, c], rc[:, c],
                                                OP.mult)

            for u in range(n_kt + 1):
                units.append(lambda u=u: step(u))
            units.append(flush)
            return units

        ob_tiles = {}

        def oproj_units(qc, tailq=False):
            """16 units: (qt4, hcn) -> 2 matmuls + evict; DMA per qt4."""
            units = []
            for qt4 in range(4):
                for hcn in range(4):
                    def grp(qc=qc, qt4=qt4, hcn=hcn, tailq=tailq):
                        if hcn == 0:
                            ob_tiles[(qc, qt4)] = obpool.tile(
                                [128, H], bf16, tag="ob", name="ob")
                        ob = ob_tiles[(qc, qt4)]
                        po = psum_pools["pop"].tile([128, 512], f32, tag="po",
                                                    name="po")
                        for h in range(HPC):
                            nc.tensor.matmul(
                                po[:], ot_sb[h][qc][:, qt4 * 128:(qt4 + 1) * 128],
                                wo_sb[:, h, hcn * 512:(hcn + 1) * 512],
                                start=(h == 0), stop=(h == HPC - 1))
                        dst = ob[:, hcn * 512:(hcn + 1) * 512]
                        if hcn % 2 == 0:
                            nc.scalar.copy(dst, po[:])
                        else:
                            nc.vector.tensor_copy(dst, po[:])
                        if hcn == 3:
                            qt = qc * 4 + qt4
                            engs = ([nc.sync, nc.gpsimd, nc.scalar]
                                    if tailq else [nc.sync, nc.gpsimd])
                            eng = engs[qt4 % len(engs)]
                            eng.dma_start(
                                out_d[qt * 128:(qt + 1) * 128, :], ob[:])
                            del ob_tiles[(qc, qt4)]
                    units.append(grp)
            return units

        def emit_interleaved(pe_thunks, unit_list):
            """Spread units evenly among the PE thunk stream."""
            n_t, n_u = len(pe_thunks), len(unit_list)
            ui = 0
            for i, th in enumerate(pe_thunks):
                while ui < n_u and ui * (n_t + 1) <= i * n_u:
                    unit_list[ui]()
                    ui += 1
                if isinstance(th, tuple):
                    th[0]()
                    th[1]()
                else:
                    th()
            while ui < n_u:
                unit_list[ui]()
                ui += 1

        # ---------------- emission ----------------
        # the gpsimd DMA queue measures ~1.5x the sync queue's rate, so it
        # carries 10 of the 16 x chunks; interleaved so arrivals track the
        # chains' in-order kc consumption
        g_kcs = [0, 1, 2, 4, 5, 7, 8, 10, 11, 13]
        s_kcs = [3, 6, 9, 12, 14, 15]
        emit_x_dma(s_kcs, nc.sync, 0)
        nc.sync.dma_start(sin_sb[:], sin_d[:])
        emit_x_dma(s_kcs, nc.sync, 1)
        emit_x_dma(g_kcs, nc.gpsimd, 0)
        nc.gpsimd.dma_start(cos_sb[:], cos_d[:])
        nc.gpsimd.dma_start(msk_sb[:], msk_d[:])
        nc.gpsimd.dma_start(ones_sb[:], ones_d[:])
        emit_x_dma(g_kcs, nc.gpsimd, 1)

        # lc0 in its own wide psum pool (closed before the steady-state
        # pools open): Q0/Q1/K0/K1 interleaved per kc, then V lt-major.
        with ExitStack() as lc0_scope:
            lc0_p = lc0_scope.enter_context(
                tc.tile_pool(name="lc0", bufs=8, space="PSUM"))
            qk = [(wq_sb, 0, qt_sb[0][0]), (wq_sb, 1, qt_sb[1][0]),
                  (wk_sb, 0, kt_sb[0][0]), (wk_sb, 1, kt_sb[1][0])]
            chains = [qk_chain(lc0_p, "l0", w, 0, h, dst) for w, h, dst in qk]
            for kc in range(KC):
                for ch in chains:
                    th = ch[kc]
                    if isinstance(th, tuple):
                        th[0]()
                        th[1]()
                    else:
                        th()
            for lt in range(4):
                for th in v_chain(lc0_p, "l0", 0, lt):
                    if isinstance(th, tuple):
                        th[0]()
                        th[1]()
                    else:
                        th()

        psum_pools["pj"] = top.enter_context(
            tc.tile_pool(name="pj", bufs=2, space="PSUM"))
        psum_pools["sps"] = top.enter_context(
            tc.tile_pool(name="sps", bufs=2, space="PSUM"))
        psum_pools["acc"] = top.enter_context(
            tc.tile_pool(name="acc", bufs=1, space="PSUM"))
        psum_pools["pop"] = top.enter_context(
            tc.tile_pool(name="pop", bufs=2, space="PSUM"))

        for i in range(QCN):
            units = att_units(i)
            if i >= 1:
                units = _merge(units, oproj_units(i - 1))
            if i < 3:
                emit_interleaved(proj_thunks(i + 1), units)
            else:
                emit_interleaved([], units)
        emit_interleaved([], oproj_units(3, tailq=True))

    nc.compile()
    return nc


def _merge(a, b):
    """Round-robin merge of two unit lists, proportionally."""
    out = []
    ia = ib = 0
    n = len(a) + len(b)
    for i in range(n):
        if ia * len(b) <= ib * len(a) and ia < len(a):
            out.append(a[ia]); ia += 1
        elif ib < len(b):
            out.append(b[ib]); ib += 1
        else:
            out.append(a[ia]); ia += 1
    return out


def _prep_inputs(x, Wq, Wk, Wv, Wo):
    import ml_dtypes
    bf16 = ml_dtypes.bfloat16
    xT = np.ascontiguousarray(x.reshape(L, H).T).astype(bf16).reshape(KC, 128, L)
    cosT, sinTs = _rope_tables()
    masks = _causal_masks()
    ones = np.ones((128, 128), dtype=np.float32)
    scale = np.float32(1.0 / np.sqrt(D))
    def pmajor(w):     # [H, 256] -> [128, KC, 256] partition-major
        return np.ascontiguousarray(
            w.reshape(KC, 128, HPC * D).transpose(1, 0, 2))
    in_maps = []
    for i in range(NCORES):
        rs = slice(i * HPC * D, (i + 1) * HPC * D)
        in_maps.append({
            "xT": xT,
            "wqT": pmajor((Wq[rs].T * scale).astype(bf16)),
            "wkT": pmajor(Wk[rs].T.astype(bf16)),
            "wvT": pmajor(Wv[rs].T.astype(bf16)),
            "woP": np.ascontiguousarray(
                Wo[:, rs].T.reshape(HPC, 128, H).transpose(1, 0, 2)).astype(bf16),
            "cosT": cosT.astype(bf16),
            "sinTs": sinTs.astype(bf16),
            "masks": masks.astype(bf16),
            "ones": ones,
        })
    return in_maps


def run(x, Wq, Wk, Wv, Wo, trace=False):
    from concourse.bass_utils import run_bass_kernel_spmd
    if "nc" not in _CACHE:
        _CACHE["nc"] = _build_nc()
    nc = _CACHE["nc"]
    in_maps = _prep_inputs(np.asarray(x), np.asarray(Wq), np.asarray(Wk),
                           np.asarray(Wv), np.asarray(Wo))
    res = run_bass_kernel_spmd(nc, in_maps, core_ids=list(range(NCORES)),
                               trace=trace)
    acc = np.zeros((L, H), dtype=np.float64)
    for r in res.results:
        acc += r["out"].astype(np.float64)
    return acc.astype(np.float32).reshape(1, L, H), res


def kernel(x, Wq, Wk, Wv, Wo):
    out, _ = run(x, Wq, Wk, Wv, Wo)
    return out


# revision 39
# speedup vs baseline: 1.0483x; 1.0104x over previous
"""Causal attention (RoPE, 16 heads, L=2048, H=2048) on 8 trn2 NeuronCores.

Sharding: tensor-parallel over heads. Core i handles heads 2i, 2i+1
(d=128 each): column-parallel q/k/v projections, row-parallel o_proj,
host-side sum of the 8 partial outputs.

v2: single fused instruction stream; block i interleaves projection
chunk lc=i+1 with attention q-chunk qc=i and o_proj of qc=i-1 so the
ACT-bound exp work hides under PE-bound projection matmuls.
  - Q^T/K^T in [d, L] layout (weight-stationary, bf16 in, N=512);
    RoPE on DVE reading PSUM directly.
  - V in natural [L, d] layout (x-stationary, N=256) - no transposes.
  - Causal mask preloaded into PSUM as additive -1e30 (DVE copy), the
    S^T matmul accumulates onto it with start=False; exp gives exact 0.
  - Softmax denominators: gpsimd accumulates exp tiles elementwise
    (ptsum), a single ones-matmul per (head, q-chunk) reduces over
    partitions; DVE reciprocal + multiply normalizes into ot (aliasing
    the dead qt tile).
  - o_proj per 128-row q-tile into a contiguous [128, 2048] buffer,
    one 1MB output DMA each, round-robin across queues at the tail.
"""
import numpy as np

L = 2048
H = 2048
NH = 16
D = 128          # head dim
NCORES = 8
HPC = NH // NCORES   # heads per core = 2
ROPE_BASE = 10000.0
KC = H // 128        # 16 contraction chunks
LCN = 4              # L chunks of 512
QCN = 4              # q chunks of 512

_CACHE = {}


def _rope_tables():
    inv_freq = 1.0 / (ROPE_BASE ** (np.arange(0, D, 2, dtype=np.float32) / D))
    t = np.arange(L, dtype=np.float32)
    freqs = np.outer(t, inv_freq).astype(np.float32)          # [L, D/2]
    emb = np.concatenate([freqs, freqs], axis=-1)             # [L, D]
    cos = np.cos(emb).astype(np.float32)                      # [L, D]
    sin = np.sin(emb).astype(np.float32)
    cosT = np.ascontiguousarray(cos.T)                        # [D, L]
    sinT = np.ascontiguousarray(sin.T)
    sinTs = sinT.copy()
    sinTs[: D // 2] = -sinT[: D // 2]                         # sign-folded
    # partition-swapped so DVE operand base partitions match:
    # sinsw[p] = sinTs[(p+64) % 128]
    sinsw = np.concatenate([sinTs[D // 2:], sinTs[: D // 2]], axis=0)
    return cosT, np.ascontiguousarray(sinsw)


def _causal_masks():
    # multiplicative triangle for the diagonal 128x128 block (same for
    # every j): keep (1.0) iff col >= row
    row = np.arange(128)[:, None]
    col = np.arange(128)[None, :]
    return np.where(col - row >= 0, 1.0, 0.0).astype(np.float32)


def _build_nc():
    import concourse.bacc as bacc
    import concourse.mybir as mybir
    from concourse import tile
    from contextlib import ExitStack

    f32 = mybir.dt.float32
    f32r = mybir.dt.float32r
    bf16 = mybir.dt.bfloat16
    AF = mybir.ActivationFunctionType
    OP = mybir.AluOpType

    nc = bacc.Bacc("TRN2", target_bir_lowering=False, debug=False)

    # weights host-rearranged to partition-major so DMAs are contiguous
    xT_d = nc.dram_tensor("xT", (KC, 128, L), bf16, kind="ExternalInput")
    wq_d = nc.dram_tensor("wqT", (128, KC, HPC * D), bf16, kind="ExternalInput")
    wk_d = nc.dram_tensor("wkT", (128, KC, HPC * D), bf16, kind="ExternalInput")
    wv_d = nc.dram_tensor("wvT", (128, KC, HPC * D), bf16, kind="ExternalInput")
    wo_d = nc.dram_tensor("woP", (128, HPC, H), bf16, kind="ExternalInput")
    cos_d = nc.dram_tensor("cosT", (D, L), bf16, kind="ExternalInput")
    sin_d = nc.dram_tensor("sinTs", (D, L), bf16, kind="ExternalInput")
    msk_d = nc.dram_tensor("masks", (128, 128), bf16, kind="ExternalInput")
    ones_d = nc.dram_tensor("ones", (128, 128), f32r, kind="ExternalInput")
    out_d = nc.dram_tensor("out", (L, H), bf16, kind="ExternalOutput")

    with tile.TileContext(nc) as tc, ExitStack() as top:
        per = top.enter_context(tc.tile_pool(name="per", bufs=1))

        wq_sb = per.tile([128, KC, HPC * D], bf16)
        wk_sb = per.tile([128, KC, HPC * D], bf16)
        wv_sb = per.tile([128, KC, HPC * D], bf16)
        wo_sb = per.tile([128, HPC, H], bf16)
        cos_sb = per.tile([128, L], bf16)
        sin_sb = per.tile([128, L], bf16)
        msk_sb = per.tile([128, 128], bf16)
        ones_sb = per.tile([128, 128], f32r)
        qt_sb = [[per.tile([128, 512], bf16, name=f"qt{h}_{c}")
                  for c in range(QCN)] for h in range(HPC)]
        # normalized O^T in bf16 (o_proj runs fully bf16)
        ot_sb = [[per.tile([128, 512], bf16, name=f"ot{h}_{c}")
                  for c in range(QCN)] for h in range(HPC)]
        kt_sb = [[per.tile([128, 512], bf16, name=f"kt{h}_{c}")
                  for c in range(LCN)] for h in range(HPC)]
        # natural V: [l within 128-tile, lt, d of both heads]
        v_sb = per.tile([128, KC, HPC * D], bf16, name="v")
        ptsum = [per.tile([128, 512], f32r, name=f"ptsum{h}") for h in range(HPC)]
        # all of x^T stays resident: 16 chunks x [128, 2048] bf16 = 8MB,
        # loaded once with 4KB partition lines (small lines run ~45GB/s);
        # one tile per chunk so dependencies stay per-kc
        xt_k = [per.tile([128, L], bf16, name=f"xt{kc}") for kc in range(KC)]

        ptpool = top.enter_context(tc.tile_pool(name="pt", bufs=3))
        tpool = top.enter_context(tc.tile_pool(name="tmp", bufs=1))
        obpool = top.enter_context(tc.tile_pool(name="ob", bufs=2))
        psum_pools = {}

        # ---------- initial DMAs ----------
        # Per-queue DMA runs ~90GB/s and there are only 3 queues, so every
        # transfer is ordered by its consumption deadline.  x chunks load
        # as column halves (2KB lines): lc0/lc1 half first, lc2/lc3 later.
        for a, b in ((0, 2), (2, 6), (6, 10), (10, 16)):
            nc.scalar.dma_start(wq_sb[:, a:b, :], wq_d[:, a:b, :])
            nc.scalar.dma_start(wk_sb[:, a:b, :], wk_d[:, a:b, :])
        nc.scalar.dma_start(wv_sb[:, 0:8, :], wv_d[:, 0:8, :])
        nc.scalar.dma_start(wv_sb[:, 8:16, :], wv_d[:, 8:16, :])
        nc.scalar.dma_start(wo_sb[:], wo_d[:])

        def emit_x_dma(kcs, eng, half):
            c = slice(0, 1024) if half == 0 else slice(1024, 2048)
            for kc in kcs:
                eng.dma_start(xt_k[kc][:, c], xT_d[kc, :, c])

        # ---------------- thunk builders ----------------
        def rope_evict(ps, dst, lc):
            cs = slice(lc * 512, (lc + 1) * 512)
            t1 = tpool.tile([128, 512], f32, tag="t1")
            t2 = tpool.tile([128, 512], f32, tag="t2")
            nc.vector.tensor_tensor(t2[:], ps[:], cos_sb[:, cs], OP.mult)
            nc.vector.tensor_tensor(
                t1[0:64, :], ps[64:128, :], sin_sb[64:128, cs], OP.mult)
            nc.vector.tensor_tensor(
                t1[64:128, :], ps[0:64, :], sin_sb[0:64, cs], OP.mult)
            nc.vector.tensor_tensor(dst[:], t1[:], t2[:], OP.add)

        def qk_chain(pool, tag, w_sb, lc, h, dst):
            """16 matmul thunks accumulating one head's Q^T/K^T chunk."""
            ps = pool.tile([128, 512], f32, tag=tag, name="ps")
            thunks = []
            for kc in range(KC):
                def mm(kc=kc, ps=ps, w_sb=w_sb, h=h, lc=lc):
                    nc.tensor.matmul(
                        ps[:], w_sb[:, kc, h * D:(h + 1) * D],
                        xt_k[kc][:, lc * 512:(lc + 1) * 512],
                        start=(kc == 0), stop=(kc == KC - 1))
                thunks.append(mm)
            thunks[-1] = (thunks[-1], lambda ps=ps, dst=dst, lc=lc:
                          rope_evict(ps, dst, lc))
            return thunks

        def v_chain(pool, tag, lc, lt):
            ps = pool.tile([128, 512], f32, tag=tag, name="ps")
            thunks = []
            for kc in range(KC):
                def mm(kc=kc, ps=ps, lt=lt, lc=lc):
                    c0 = lc * 512 + lt * 128
                    nc.tensor.matmul(
                        ps[:, 0:HPC * D],
                        xt_k[kc][:, c0:c0 + 128],
                        wv_sb[:, kc, :],
                        start=(kc == 0), stop=(kc == KC - 1))
                thunks.append(mm)
            def ev(ps=ps, lt=lt, lc=lc):
                nc.scalar.copy(v_sb[:, lc * 4 + lt, :], ps[:, 0:HPC * D])
            thunks[-1] = (thunks[-1], ev)
            return thunks

        def proj_thunks(lc):
            """128 PE thunks for chunk lc: Q0,K0,Q1,K1 then V (2 psum bufs).

            Q/K alternate so each chain's RoPE eviction has a full chain of
            slack before its psum slot is reused."""
            pj = psum_pools["pj"]
            thunks = []
            thunks += qk_chain(pj, "pj", wq_sb, lc, 0, qt_sb[0][lc])
            thunks += qk_chain(pj, "pj", wk_sb, lc, 0, kt_sb[0][lc])
            thunks += qk_chain(pj, "pj", wq_sb, lc, 1, qt_sb[1][lc])
            thunks += qk_chain(pj, "pj", wk_sb, lc, 1, kt_sb[1][lc])
            for lt in range(4):
                thunks += v_chain(pj, "pj", lc, lt)
            return thunks

        def att_units(qc):
            """n_kt+2 units: pipelined S/exp then PV/accum one kt behind."""
            n_kt = 4 * qc + 4
            units = []
            pts = {}    # (kt, h) -> pt tile
            acc_p = psum_pools["acc"]
            sps_p = psum_pools["sps"]
            sums = [acc_p.tile([128, 512], f32, tag=f"o{h}", name=f"ops{h}")
                    for h in range(HPC)]

            def step(u, qc=qc, n_kt=n_kt):
                if u < n_kt:
                    kt = u
                    diag = kt >= 4 * qc
                    # diagonal tiles: valid cols >= 128j.  Compute S unmasked
                    # on [128j:512] (bf16 has no minimum-N penalty), exp it,
                    # then zero the triangle block [128j:128j+128) of pt with
                    # one cheap bf16 multiply and memset the dead columns.
                    j = kt - 4 * qc
                    c0 = 128 * j if diag else 0
                    for h in range(HPC):
                        s = sps_p.tile([128, 512], f32, tag="s")
                        nc.tensor.matmul(
                            s[:, c0:],
                            kt_sb[h][kt // 4][:, (kt % 4) * 128:(kt % 4 + 1) * 128],
                            qt_sb[h][qc][:, c0:],
                            start=True, stop=True)
                        pt = ptpool.tile([128, 512], bf16, tag=f"pt{h}")
                        if c0:
                            nc.vector.memset(pt[:, 0:c0], 0.0)
                        nc.scalar.activation(pt[:, c0:], s[:, c0:], AF.Exp)
                        if diag:
                            tb = slice(128 * j, 128 * (j + 1))
                            nc.vector.tensor_tensor(
                                pt[:, tb], pt[:, tb], msk_sb[:], OP.mult)
                        pts[(kt, h)] = pt
                if u >= 1:
                    kt = u - 1
                    for h in range(HPC):
                        nc.tensor.matmul(
                            sums[h][:], v_sb[:, kt, h * D:(h + 1) * D],
                            pts[(kt, h)][:],
                            start=(kt == 0), stop=(kt == n_kt - 1))
                    for h in range(HPC):
                        # h0 on DVE, h1 on gpsimd; last kt both on DVE (it
                        # sits on the flush critical path and DVE is faster)
                        eng = nc.vector if (h == 0 or kt == n_kt - 1) \
                            else nc.gpsimd
                        if kt == 0:
                            eng.tensor_copy(ptsum[h][:], pts[(kt, h)][:])
                        else:
                            eng.tensor_tensor(
                                ptsum[h][:], ptsum[h][:], pts[(kt, h)][:],
                                OP.add)
                        del pts[(kt, h)]

            def flush(qc=qc):
                for h in range(HPC):
                    den = sps_p.tile([128, 512], f32, tag="s")
                    nc.tensor.matmul(den[:], ones_sb[:], ptsum[h][:],
                                     start=True, stop=True)
                    rc = tpool.tile([128, 512], f32, tag="rc")
                    nc.vector.reciprocal_approx_fast(rc[:], den[:])
                    # last chunk sliced so o_proj can start on slice 0 early
                    nsl = 4 if qc == QCN - 1 else 1
                    for sl in range(nsl):
                        c = slice(sl * 512 // nsl, (sl + 1) * 512 // nsl)
                        nc.vector.tensor_tensor(ot_sb[h][qc][:, c],
                                                sums[h][:, c], rc[:, c],
                                                OP.mult)

            for u in range(n_kt + 1):
                units.append(lambda u=u: step(u))
            units.append(flush)
            return units

        ob_tiles = {}

        def oproj_units(qc, tailq=False):
            """16 units: (qt4, hcn) -> 2 matmuls + evict; DMA per qt4."""
            units = []
            for qt4 in range(4):
                for hcn in range(4):
                    def grp(qc=qc, qt4=qt4, hcn=hcn, tailq=tailq):
                        if hcn == 0:
                            ob_tiles[(qc, qt4)] = obpool.tile(
                                [128, H], bf16, tag="ob", name="ob")
                        ob = ob_tiles[(qc, qt4)]
                        po = psum_pools["pop"].tile([128, 512], f32, tag="po",
                                                    name="po")
                        for h in range(HPC):
                            nc.tensor.matmul(
                                po[:], ot_sb[h][qc][:, qt4 * 128:(qt4 + 1) * 128],
                                wo_sb[:, h, hcn * 512:(hcn + 1) * 512],
                                start=(h == 0), stop=(h == HPC - 1))
                        dst = ob[:, hcn * 512:(hcn + 1) * 512]
                        if hcn % 2 == 0:
                            nc.scalar.copy(dst, po[:])
                        else:
                            nc.vector.tensor_copy(dst, po[:])
                        if hcn == 3:
                            qt = qc * 4 + qt4
                            engs = ([nc.sync, nc.gpsimd, nc.scalar]
                                    if tailq else [nc.sync, nc.gpsimd])
                            eng = engs[qt4 % len(engs)]
                            eng.dma_start(
                                out_d[qt * 128:(qt + 1) * 128, :], ob[:])
                            del ob_tiles[(qc, qt4)]
                    units.append(grp)
            return units

        def emit_interleaved(pe_thunks, unit_list):
            """Spread units evenly among the PE thunk stream."""
            n_t, n_u = len(pe_thunks), len(unit_list)
            ui = 0
            for i, th in enumerate(pe_thunks):
                while ui < n_u and ui * (n_t + 1) <= i * n_u:
                    unit_list[ui]()
                    ui += 1
                if isinstance(th, tuple):
                    th[0]()
                    th[1]()
                else:
                    th()
            while ui < n_u:
                unit_list[ui]()
                ui += 1

        # ---------------- emission ----------------
        # the gpsimd DMA queue measures ~1.5x the sync queue's rate, so it
        # carries 10 of the 16 x chunks; interleaved so arrivals track the
        # chains' in-order kc consumption
        g_kcs = [0, 1, 2, 4, 5, 7, 8, 10, 11, 13]
        s_kcs = [3, 6, 9, 12, 14, 15]
        emit_x_dma(s_kcs, nc.sync, 0)
        nc.sync.dma_start(sin_sb[:], sin_d[:])
        emit_x_dma(s_kcs, nc.sync, 1)
        emit_x_dma(g_kcs, nc.gpsimd, 0)
        nc.gpsimd.dma_start(cos_sb[:], cos_d[:])
        nc.gpsimd.dma_start(msk_sb[:], msk_d[:])
        nc.gpsimd.dma_start(ones_sb[:], ones_d[:])
        emit_x_dma(g_kcs, nc.gpsimd, 1)

        # lc0 in its own wide psum pool (closed before the steady-state
        # pools open): Q0/Q1/K0/K1 interleaved per kc, then V lt-major.
        with ExitStack() as lc0_scope:
            lc0_p = lc0_scope.enter_context(
                tc.tile_pool(name="lc0", bufs=8, space="PSUM"))
            qk = [(wq_sb, 0, qt_sb[0][0]), (wq_sb, 1, qt_sb[1][0]),
                  (wk_sb, 0, kt_sb[0][0]), (wk_sb, 1, kt_sb[1][0])]
            chains = [qk_chain(lc0_p, "l0", w, 0, h, dst) for w, h, dst in qk]
            for kc in range(KC):
                for ch in chains:
                    th = ch[kc]
                    if isinstance(th, tuple):
                        th[0]()
                        th[1]()
                    else:
                        th()
            for lt in range(4):
                for th in v_chain(lc0_p, "l0", 0, lt):
                    if isinstance(th, tuple):
                        th[0]()
                        th[1]()
                    else:
                        th()

        psum_pools["pj"] = top.enter_context(
            tc.tile_pool(name="pj", bufs=2, space="PSUM"))
        psum_pools["sps"] = top.enter_context(
            tc.tile_pool(name="sps", bufs=2, space="PSUM"))
        psum_pools["acc"] = top.enter_context(
            tc.tile_pool(name="acc", bufs=1, space="PSUM"))
        psum_pools["pop"] = top.enter_context(
            tc.tile_pool(name="pop", bufs=2, space="PSUM"))

        for i in range(QCN):
            units = att_units(i)
            if i >= 1:
                units = _merge(units, oproj_units(i - 1))
            if i < 3:
                emit_interleaved(proj_thunks(i + 1), units)
            else:
                emit_interleaved([], units)
        emit_interleaved([], oproj_units(3, tailq=True))

    nc.compile()
    return nc


def _merge(a, b):
    """Round-robin merge of two unit lists, proportionally."""
    out = []
    ia = ib = 0
    n = len(a) + len(b)
    for i in range(n):
        if ia * len(b) <= ib * len(a) and ia < len(a):
            out.append(a[ia]); ia += 1
        elif ib < len(b):
            out.append(b[ib]); ib += 1
        else:
            out.append(a[ia]); ia += 1
    return out


def _prep_inputs(x, Wq, Wk, Wv, Wo):
    import ml_dtypes
    bf16 = ml_dtypes.bfloat16
    xT = np.ascontiguousarray(x.reshape(L, H).T).astype(bf16).reshape(KC, 128, L)
    cosT, sinTs = _rope_tables()
    masks = _causal_masks()
    ones = np.ones((128, 128), dtype=np.float32)
    scale = np.float32(1.0 / np.sqrt(D))
    def pmajor(w):     # [H, 256] -> [128, KC, 256] partition-major
        return np.ascontiguousarray(
            w.reshape(KC, 128, HPC * D).transpose(1, 0, 2))
    in_maps = []
    for i in range(NCORES):
        rs = slice(i * HPC * D, (i + 1) * HPC * D)
        in_maps.append({
            "xT": xT,
            "wqT": pmajor((Wq[rs].T * scale).astype(bf16)),
            "wkT": pmajor(Wk[rs].T.astype(bf16)),
            "wvT": pmajor(Wv[rs].T.astype(bf16)),
            "woP": np.ascontiguousarray(
                Wo[:, rs].T.reshape(HPC, 128, H).transpose(1, 0, 2)).astype(bf16),
            "cosT": cosT.astype(bf16),
            "sinTs": sinTs.astype(bf16),
            "masks": masks.astype(bf16),
            "ones": ones,
        })
    return in_maps


def run(x, Wq, Wk, Wv, Wo, trace=False):
    from concourse.bass_utils import run_bass_kernel_spmd
    if "nc" not in _CACHE:
        _CACHE["nc"] = _build_nc()
    nc = _CACHE["nc"]
    in_maps = _prep_inputs(np.asarray(x), np.asarray(Wq), np.asarray(Wk),
                           np.asarray(Wv), np.asarray(Wo))
    res = run_bass_kernel_spmd(nc, in_maps, core_ids=list(range(NCORES)),
                               trace=trace)
    acc = np.zeros((L, H), dtype=np.float64)
    for r in res.results:
        acc += r["out"].astype(np.float64)
    return acc.astype(np.float32).reshape(1, L, H), res


def kernel(x, Wq, Wk, Wv, Wo):
    out, _ = run(x, Wq, Wk, Wv, Wo)
    return out
